# revision 5
# baseline (speedup 1.0000x reference)
"""Trainium2 Bass kernel for nn_AttentionQKV (causal attention + GCN refinement).

Sharding: batch*heads across 8 cores (core c: batch c//4, heads [4*(c%4), 4*(c%4)+4)).
The final Wo projection is partial-summed with a ReduceScatter over each batch's 4
cores, followed by the output layernorm; each core writes a 512-row slice.

Per-core math (head h, E = exp(QK^T/sqrt(d)) causal, D_i = sum_j E_ij):
  attn = D^-1 E;  adj = attn with diag set to 1;  deg_i = 2 - attn_ii
  (adj@h)_i = (Eh)_i/D_i + (1-attn_ii) h_i     (diag correction, adj never built)
  E@h1 = (E@V)@W1 + D b1^T                     (layer-1 linearity: h1 = V@W1+b1)
  D comes free as a 65th ones-column in the [V|1] pass-A stationary.
All row-softmax scalars live in [token%128, tile] column layout so vector ops use
all 128 lanes; broadcast row-matrices are materialized via a DRAM round-trip
(step-0 partition APs are only legal for DRAM-source DMA).
"""

import os
import numpy as np

import concourse.bass as bass
import concourse.tile as tile
import concourse.mybir as mybir
from concourse import bacc
from concourse.bass_utils import run_bass_kernel_spmd
from concourse.masks import make_identity

dt = mybir.dt
F32 = dt.float32
F32R = dt.float32r
BF16 = dt.bfloat16
AF = mybir.ActivationFunctionType
ALU = mybir.AluOpType

B, N, DIM = 2, 2048, 1024
HEADS, DHEAD = 16, 64
HPC = 4                 # heads per core
EPS = 1e-5
NT = N // 128           # 16 token tiles
KT8 = DIM // 128        # 8 dim tiles
SCALE = DHEAD ** -0.5

# column offset of j-tile jt inside the packed E^T tile (width of jt = N-128*jt)
ET_OFF = [0]
for _jt in range(1, NT):
    ET_OFF.append(ET_OFF[-1] + (N - 128 * (_jt - 1)))
ET_TOT = ET_OFF[-1] + (N - 128 * (NT - 1))  # 17408


def _bc(ap, parts):
    """broadcast a DRAM AP across `parts` partitions (prepend step-0 dim)."""
    dims = list(ap.ap)
    if dims and dims[0][1] == 1:
        dims = dims[1:]
    return bass.AP(tensor=ap.tensor, offset=ap.offset, ap=[[0, parts]] + dims)


def _row_of_cols(row_ap):
    """view a [1, N] DRAM row AP in (token%128)-outer, tile-inner iteration
    [[1,128],[128,ntiles]] so it can pair with a [128, ntiles] SBUF column tile."""
    n = row_ap.ap[-1][1]
    return bass.AP(tensor=row_ap.tensor, offset=row_ap.offset,
                   ap=[[1, 128], [128, n // 128]])


def build_program(has_gnn_b, has_lin_b, gin_ones, gout_ones):
    nc = bacc.Bacc("TRN2", target_bir_lowering=False, debug=False, num_devices=8)

    x_in = nc.dram_tensor("x_in", [N, DIM], F32, kind="ExternalInput").ap()
    wq_in = nc.dram_tensor("wq_in", [DIM, 256], F32R, kind="ExternalInput").ap()
    wk_in = nc.dram_tensor("wk_in", [DIM, 256], F32R, kind="ExternalInput").ap()
    wv_in = nc.dram_tensor("wv_in", [DIM, 256], F32R, kind="ExternalInput").ap()
    wo_in = nc.dram_tensor("wo_in", [256, DIM], BF16, kind="ExternalInput").ap()
    gin_in = nc.dram_tensor("gin_in", [1, DIM], F32, kind="ExternalInput").ap()
    gout_in = nc.dram_tensor("gout_in", [1, DIM], F32, kind="ExternalInput").ap()
    gnw_in = nc.dram_tensor("gnw_in", [2, DHEAD, DHEAD], BF16, kind="ExternalInput").ap()
    gnb_in = nc.dram_tensor("gnb_in", [2, DHEAD], F32, kind="ExternalInput").ap()
    linw_in = nc.dram_tensor("linw_in", [DHEAD, DHEAD], BF16, kind="ExternalInput").ap()
    linb_in = nc.dram_tensor("linb_in", [1, DHEAD], F32, kind="ExternalInput").ap()
    y_out = nc.dram_tensor("y_out", [512, DIM], F32, kind="ExternalOutput").ap()

    cc_in = nc.dram_tensor("cc_in", [N, DIM], F32).ap()
    cc_out = nc.dram_tensor("cc_out", [512, DIM], F32).ap()
    sc_dram = nc.dram_tensor("sc_dram", [HPC, 3, N], BF16).ap()   # invD, a, b rows
    sii_dram = nc.dram_tensor("sii_dram", [HPC, N], F32).ap()
    d_dram = nc.dram_tensor("d_dram", [HPC, N], F32).ap()
    da_dram = nc.dram_tensor("da_dram", [HPC, N], BF16).ap() if has_gnn_b else None

    groups = [[0, 1, 2, 3], [4, 5, 6, 7]]

    with tile.TileContext(nc) as tc:
        from contextlib import ExitStack
        with ExitStack() as ctx:
            const = ctx.enter_context(tc.tile_pool(name="const", bufs=1))
            persist = ctx.enter_context(tc.tile_pool(name="persist", bufs=1))

            ident32 = const.tile([128, 128], F32)
            make_identity(nc, ident32[:])
            ident_r = const.tile([128, 128], F32R)
            nc.vector.tensor_copy(out=ident_r[:], in_=ident32[:])
            ident_b = const.tile([128, 128], BF16)
            nc.vector.tensor_copy(out=ident_b[:], in_=ident32[:])
            ones_r = const.tile([128, 1], F32R)
            ones32 = const.tile([128, 1], F32)
            nc.vector.memset(ones32[:], 1.0)
            nc.vector.tensor_copy(out=ones_r[:], in_=ones32[:])
            eps_t = const.tile([128, 1], F32)
            nc.vector.memset(eps_t[:], EPS)

            wo_sb = const.tile([128, 2, DIM], BF16)
            nc.sync.dma_start(out=wo_sb[:], in_=wo_in.rearrange("(k p) d -> p k d", p=128))
            gnw_sb = const.tile([DHEAD, 2, DHEAD], BF16)
            nc.sync.dma_start(out=gnw_sb[:], in_=gnw_in.rearrange("l k d -> k l d"))
            linw_sb = const.tile([DHEAD, DHEAD], BF16)
            nc.sync.dma_start(out=linw_sb[:], in_=linw_in)
            gnb_sb = linb_sb = None
            if has_gnn_b:
                gnb_sb = const.tile([DHEAD, 2], F32)   # per-partition cols (d, l)
                nc.sync.dma_start(out=gnb_sb[:], in_=bass.AP(
                    tensor=gnb_in.tensor, offset=gnb_in.offset, ap=[[1, DHEAD], [DHEAD, 2]]))
            if has_lin_b:
                linb_sb = const.tile([DHEAD, 1], F32)
                nc.sync.dma_start(out=linb_sb[:], in_=bass.AP(
                    tensor=linb_in.tensor, offset=linb_in.offset, ap=[[1, DHEAD], [DHEAD, 1]]))
            gin_col = None
            if not gin_ones:
                gin_col = const.tile([128, KT8], F32)
                nc.sync.dma_start(out=gin_col[:], in_=bass.AP(
                    tensor=gin_in.tensor, offset=gin_in.offset, ap=[[1, 128], [128, KT8]]))
            gout_mat = None
            if not gout_ones:
                gout_mat = const.tile([128, DIM], F32)
                nc.sync.dma_start(out=gout_mat[:], in_=_bc(gout_in, 128))

            # persistent tensors
            qt = [persist.tile([128, N], F32R, name=f"qt{p}") for p in range(2)]
            kt = [persist.tile([128, N], F32R, name=f"kt{p}") for p in range(2)]
            vt1 = persist.tile([128, NT, HPC, 65], BF16)      # [V_h | 1] token layout
            out_pair = [persist.tile([128, N], BF16, name=f"op{p}") for p in range(2)]
            eii_c = persist.tile([128, HPC, NT], F32)         # exp(Sii) columns

            # ---------- Phase 1+2: LN -> xnT -> QKV projections ----------
            with tc.tile_pool(name="proj", bufs=1) as proj, \
                 tc.tile_pool(name="ph1", bufs=3) as ph1, \
                 tc.tile_pool(name="ph1ps", bufs=2, space="PSUM") as ph1ps:
                wq_sb = proj.tile([128, KT8, 256], F32R, name="wq_sb")
                wk_sb = proj.tile([128, KT8, 256], F32R, name="wk_sb")
                wv_sb = proj.tile([128, KT8, 256], F32R, name="wv_sb")
                nc.sync.dma_start(out=wq_sb[:], in_=wq_in.rearrange("(k p) d -> p k d", p=128))
                nc.sync.dma_start(out=wk_sb[:], in_=wk_in.rearrange("(k p) d -> p k d", p=128))
                nc.sync.dma_start(out=wv_sb[:], in_=wv_in.rearrange("(k p) d -> p k d", p=128))
                xnT = proj.tile([128, KT8, N], F32R, name="xnT")

                for it in range(NT):
                    xt = ph1.tile([128, DIM], F32, name="xt")
                    nc.sync.dma_start(out=xt[:], in_=x_in[it * 128:(it + 1) * 128, :])
                    st = ph1.tile([128, 2, nc.vector.BN_STATS_DIM], F32, name="st")
                    for sg in range(2):
                        nc.vector.bn_stats(out=st[:, sg, :], in_=xt[:, sg * 512:(sg + 1) * 512])
                    mv = ph1.tile([128, nc.vector.BN_AGGR_DIM], F32, name="mv")
                    nc.vector.bn_aggr(out=mv[:], in_=st[:])
                    rstd = ph1.tile([128, 1], F32, name="rstd")
                    nc.scalar.activation(out=rstd[:], in_=mv[:, 1:2], func=AF.Sqrt, bias=eps_t[:])
                    nc.vector.reciprocal(out=rstd[:], in_=rstd[:])
                    xnt = ph1.tile([128, DIM], F32R, name="xnt")
                    nc.vector.tensor_scalar(out=xnt[:], in0=xt[:], scalar1=mv[:, 0:1],
                                            scalar2=rstd[:], op0=ALU.subtract, op1=ALU.mult)
                    for half in range(2):
                        ps = ph1ps.tile([128, 512], F32R, name="trps")
                        for q in range(4):
                            d8 = half * 4 + q
                            nc.tensor.transpose(ps[:, q * 128:(q + 1) * 128],
                                                xnt[:, d8 * 128:(d8 + 1) * 128], ident_r[:])
                        for q in range(4):
                            d8 = half * 4 + q
                            dst = xnT[:, d8, it * 128:(it + 1) * 128]
                            src = ps[:, q * 128:(q + 1) * 128]
                            if q % 2 == 0:
                                if gin_ones:
                                    nc.scalar.copy(out=dst, in_=src.bitcast(F32))
                                else:
                                    nc.scalar.activation(out=dst, in_=src.bitcast(F32),
                                                         func=AF.Copy, scale=gin_col[:, d8:d8 + 1])
                            else:
                                if gin_ones:
                                    nc.vector.tensor_copy(out=dst, in_=src)
                                else:
                                    nc.vector.tensor_scalar_mul(out=dst, in0=src.bitcast(F32),
                                                                scalar1=gin_col[:, d8:d8 + 1])

                # QT/KT per head pair (QT pre-scaled by 1/sqrt(d))
                for p in range(2):
                    for nch in range(4):
                        sl = slice(nch * 512, (nch + 1) * 512)
                        psq = ph1ps.tile([128, 512], F32, name="psq")
                        psk = ph1ps.tile([128, 512], F32, name="psk")
                        for kk in range(KT8):
                            nc.tensor.matmul(psq[:], wq_sb[:, kk, p * 128:(p + 1) * 128],
                                             xnT[:, kk, sl], start=(kk == 0), stop=(kk == KT8 - 1))
                        for kk in range(KT8):
                            nc.tensor.matmul(psk[:], wk_sb[:, kk, p * 128:(p + 1) * 128],
                                             xnT[:, kk, sl], start=(kk == 0), stop=(kk == KT8 - 1))
                        nc.scalar.activation(out=qt[p][:, sl], in_=psq[:], func=AF.Copy, scale=SCALE)
                        nc.scalar.copy(out=kt[p][:, sl], in_=psk[:])
                # V (token layout, bf16, with ones column)
                for it in range(NT):
                    psv = ph1ps.tile([128, 256], F32, name="psv")
                    for kk in range(KT8):
                        nc.tensor.matmul(psv[:], xnT[:, kk, it * 128:(it + 1) * 128],
                                         wv_sb[:, kk, :], start=(kk == 0), stop=(kk == KT8 - 1))
                    nc.vector.tensor_copy(out=vt1[:, it, :, 0:64],
                                          in_=psv[:].rearrange("p (h d) -> p h d", h=HPC))
                nc.gpsimd.memset(vt1[:, :, :, 64:65], 1.0)

            # ---------- Phase 2.5: Sii -> Eii (column layout) ----------
            with tc.tile_pool(name="sii", bufs=2) as sip, \
                 tc.tile_pool(name="siips", bufs=4, space="PSUM") as sips:
                sii_c = sip.tile([128, HPC, NT], F32, name="sii_c")
                for p in range(2):
                    qk = sip.tile([128, N], F32R, name="qk")
                    nc.vector.tensor_tensor(out=qk[:], in0=qt[p].bitcast(F32)[:],
                                            in1=kt[p].bitcast(F32)[:], op=ALU.mult)
                    for hh in range(2):
                        h = p * 2 + hh
                        hsl = slice(hh * 64, hh * 64 + 64)
                        for nch in range(4):
                            sl = slice(nch * 512, (nch + 1) * 512)
                            ps = sips.tile([1, 512], F32, name="siips")
                            nc.tensor.matmul(ps[:], ones_r[hsl, :], qk[hsl, sl],
                                             start=True, stop=True, tile_position=(hh * 64, 0))
                            sst = sip.tile([1, 512], F32, name="sst")
                            nc.scalar.copy(out=sst[:], in_=ps[:])
                            nc.sync.dma_start(out=sii_dram[h:h + 1, sl], in_=sst[:])
                for h in range(HPC):
                    nc.sync.dma_start(out=sii_c[:, h, :], in_=_row_of_cols(sii_dram[h:h + 1, :]))
                nc.scalar.activation(out=eii_c[:], in_=sii_c[:], func=AF.Exp)

            # ---------- Phase 3: per-head attention + GNN ----------
            for h in range(HPC):
                _head(tc, nc, h, qt, kt, vt1, out_pair, eii_c,
                      gnw_sb, gnb_sb, linw_sb, linb_sb, sc_dram, da_dram, d_dram,
                      ident_b, has_gnn_b, has_lin_b)

            # ---------- Phase 4: Wo + ReduceScatter + output LN ----------
            with tc.tile_pool(name="ph4", bufs=3) as ph4, \
                 tc.tile_pool(name="ph4ps", bufs=4, space="PSUM") as ph4ps:
                for it in range(NT):
                    yt = ph4.tile([128, DIM], F32, name="yt")
                    for ch in range(2):
                        ps = ph4ps.tile([128, 512], F32, name="wops")
                        sl = slice(ch * 512, (ch + 1) * 512)
                        for kk in range(2):
                            nc.tensor.matmul(ps[:], out_pair[kk][:, it * 128:(it + 1) * 128],
                                             wo_sb[:, kk, sl], start=(kk == 0), stop=(kk == 1))
                        if ch == 0:
                            nc.scalar.copy(out=yt[:, sl], in_=ps[:])
                        else:
                            nc.vector.tensor_copy(out=yt[:, sl], in_=ps[:])
                    nc.sync.dma_start(out=cc_in[it * 128:(it + 1) * 128, :], in_=yt[:])
                nc.gpsimd.collective_compute(
                    "ReduceScatter", ALU.add, replica_groups=groups,
                    ins=[cc_in], outs=[cc_out])
                for ot in range(4):
                    zt = ph4.tile([128, DIM], F32, name="zt")
                    nc.sync.dma_start(out=zt[:], in_=cc_out[ot * 128:(ot + 1) * 128, :])
                    st = ph4.tile([128, 2, nc.vector.BN_STATS_DIM], F32, name="st4")
                    for sg in range(2):
                        nc.vector.bn_stats(out=st[:, sg, :], in_=zt[:, sg * 512:(sg + 1) * 512])
                    mv = ph4.tile([128, nc.vector.BN_AGGR_DIM], F32, name="mv4")
                    nc.vector.bn_aggr(out=mv[:], in_=st[:])
                    rstd = ph4.tile([128, 1], F32, name="rstd4")
                    nc.scalar.activation(out=rstd[:], in_=mv[:, 1:2], func=AF.Sqrt, bias=eps_t[:])
                    nc.vector.reciprocal(out=rstd[:], in_=rstd[:])
                    ot_t = ph4.tile([128, DIM], F32, name="ot_t")
                    nc.vector.tensor_scalar(out=ot_t[:], in0=zt[:], scalar1=mv[:, 0:1],
                                            scalar2=rstd[:], op0=ALU.subtract, op1=ALU.mult)
                    if not gout_ones:
                        nc.vector.tensor_tensor(out=ot_t[:], in0=ot_t[:], in1=gout_mat[:],
                                                op=ALU.mult)
                    nc.sync.dma_start(out=y_out[ot * 128:(ot + 1) * 128, :], in_=ot_t[:])

    nc.compile()
    return nc


def _head(tc, nc, h, qt, kt, vt1, out_pair, eii_c,
          gnw_sb, gnb_sb, linw_sb, linb_sb, sc_dram, da_dram, d_dram,
          ident_b, has_gnn_b, has_lin_b):
    p, hh = divmod(h, 2)
    hsl = slice(hh * 64, hh * 64 + 64)

    with tc.tile_pool(name="et", bufs=1) as etp, \
         tc.tile_pool(name="hw", bufs=1) as hw, \
         tc.tile_pool(name="hw2", bufs=2) as hw2:
        et = etp.tile([128, ET_TOT], BF16, name="et")

        # --- S^T + exp (+ causal mask on diagonal blocks) ---
        with tc.tile_pool(name="stps", bufs=4, space="PSUM") as stps:
            for jt in range(NT):
                width = N - 128 * jt
                for ch in range((width + 511) // 512):
                    cw = min(512, width - ch * 512)
                    isl = slice(128 * jt + ch * 512, 128 * jt + ch * 512 + cw)
                    ps = stps.tile([128, 512], F32, name="stp")
                    nc.tensor.matmul(ps[:, 0:cw], kt[p][hsl, 128 * jt:128 * (jt + 1)],
                                     qt[p][hsl, isl], start=True, stop=True,
                                     tile_position=(hh * 64, 0))
                    nc.scalar.activation(
                        out=et[:, ET_OFF[jt] + ch * 512:ET_OFF[jt] + ch * 512 + cw],
                        in_=ps[:, 0:cw], func=AF.Exp)
                nc.gpsimd.affine_select(
                    out=et[:, ET_OFF[jt]:ET_OFF[jt] + 128],
                    in_=et[:, ET_OFF[jt]:ET_OFF[jt] + 128],
                    compare_op=ALU.is_ge, fill=0.0, base=0,
                    pattern=[[1, 128]], channel_multiplier=-1)

        evt = hw.tile([64, N], BF16, name="evt")
        d_c = hw.tile([128, NT], F32, name="d_c")

        # --- pass A: [V_h | 1]^T E^T -> [EV; D] ---
        with tc.tile_pool(name="psA", bufs=2, space="PSUM") as psAp:
            for ich in range(4):
                i0 = ich * 512
                ps = psAp.tile([128, 512], F32, name="psA")
                njt = min(NT, (ich + 1) * 4)
                for jt in range(njt):
                    rel = max(0, 128 * jt - i0)
                    ecol = ET_OFF[jt] + (i0 + rel - 128 * jt)
                    nc.tensor.matmul(ps[0:65, rel:512], vt1[:, jt, h, :],
                                     et[:, ecol:ecol + (512 - rel)],
                                     start=(jt == 0), stop=(jt == njt - 1))
                nc.scalar.copy(out=evt[:, i0:i0 + 512], in_=ps[0:64, :])
                dstage = hw2.tile([65, 512], F32, name="dstage")
                nc.scalar.copy(out=dstage[64:65, :], in_=ps[64:65, :])
                nc.sync.dma_start(out=d_dram[h:h + 1, i0:i0 + 512], in_=dstage[64:65, :])
            nc.sync.dma_start(out=d_c[:], in_=_row_of_cols(d_dram[h:h + 1, :]))

        # --- per-token scalars (column layout [128, NT]) ---
        invd_c = hw.tile([128, NT], F32, name="invd_c")
        nc.vector.reciprocal(out=invd_c[:], in_=d_c[:])
        s_c = hw.tile([128, NT], F32, name="s_c")
        nc.vector.tensor_tensor(out=s_c[:], in0=eii_c[:, h, :], in1=invd_c[:], op=ALU.mult)
        deg_c = hw.tile([128, NT], F32, name="deg_c")
        nc.vector.tensor_scalar(out=deg_c[:], in0=s_c[:], scalar1=-1.0, scalar2=2.0,
                                op0=ALU.mult, op1=ALU.add)
        nc.vector.reciprocal(out=deg_c[:], in_=deg_c[:])
        abi_c = hw.tile([128, 3, NT], BF16, name="abi_c")   # invD, a, b (bf16 cols)
        nc.vector.tensor_copy(out=abi_c[:, 0, :], in_=invd_c[:])
        nc.vector.tensor_tensor(out=abi_c[:, 1, :], in0=invd_c[:], in1=deg_c[:], op=ALU.mult)
        b_f = hw.tile([128, NT], F32, name="b_f")
        nc.vector.tensor_scalar(out=b_f[:], in0=s_c[:], scalar1=-1.0, scalar2=1.0,
                                op0=ALU.mult, op1=ALU.add)
        nc.vector.tensor_tensor(out=abi_c[:, 2, :], in0=b_f[:], in1=deg_c[:], op=ALU.mult)
        # columns -> DRAM rows -> broadcast matrices [64, 3, N] (bf16)
        for r in range(3):
            dst = bass.AP(tensor=sc_dram.tensor, offset=sc_dram.offset + (h * 3 + r) * N,
                          ap=[[1, 128], [128, NT]])
            nc.sync.dma_start(out=dst, in_=abi_c[:, r, :])
        mats = hw2.tile([64, 3, N], BF16, name="mats")
        nc.sync.dma_start(out=mats[:], in_=_bc(sc_dram[h], 64))
        invd_m, a_m, b_m = mats[:, 0, :], mats[:, 1, :], mats[:, 2, :]

        # --- V^T for this head (PE transposes of vt1 slices) ---
        vT = hw.tile([64, N], BF16, name="vT")
        with tc.tile_pool(name="vtps", bufs=2, space="PSUM") as vtps:
            for quad in range(4):
                ps = vtps.tile([64, 512], BF16, name="vtp")
                for q in range(4):
                    it = quad * 4 + q
                    nc.tensor.transpose(ps[:, q * 128:(q + 1) * 128],
                                        vt1[:, it, h, 0:64], ident_b[:])
                nc.vector.tensor_copy(out=vT[:, quad * 512:(quad + 1) * 512], in_=ps[:])

        # --- GNN layer 1 ---
        with tc.tile_pool(name="smps", bufs=2, space="PSUM") as smps:
            # w = (W1^T V^T + b1) * b + V^T
            w_t = hw.tile([64, N], F32, name="w_t")
            for ch in range(4):
                sl = slice(ch * 512, (ch + 1) * 512)
                ps = smps.tile([64, 512], F32, name="smp")
                nc.tensor.matmul(ps[:], gnw_sb[:, 0, :], vT[:, sl], start=True, stop=True)
                if has_gnn_b:
                    nc.vector.tensor_scalar(out=ps[:], in0=ps[:], scalar1=gnb_sb[:, 0:1],
                                            op0=ALU.add)
                nc.vector.tensor_tensor(out=w_t[:, sl], in0=ps[:], in1=b_m[:, sl], op=ALU.mult)
            nc.vector.tensor_tensor(out=w_t[:], in0=w_t[:], in1=vT[:], op=ALU.add)
            # t = (W1^T (EV)^T) * a + w   (+ b1 ⊗ (D*a) when gnn_b != 0)
            t_t = hw.tile([64, N], F32, name="t_t")
            for ch in range(4):
                sl = slice(ch * 512, (ch + 1) * 512)
                ps = smps.tile([64, 512], F32, name="smp")
                nc.tensor.matmul(ps[:], gnw_sb[:, 0, :], evt[:, sl], start=True, stop=True)
                nc.vector.tensor_tensor(out=t_t[:, sl], in0=ps[:], in1=a_m[:, sl], op=ALU.mult)
            if has_gnn_b:
                da_c = hw.tile([128, NT], BF16, name="da_c")
                nc.vector.tensor_tensor(out=da_c[:], in0=d_c[:], in1=abi_c[:, 1, :], op=ALU.mult)
                nc.sync.dma_start(out=bass.AP(tensor=da_dram.tensor, offset=da_dram.offset + h * N,
                                              ap=[[1, 128], [128, NT]]), in_=da_c[:])
                damat = hw.tile([64, N], BF16, name="damat")
                nc.sync.dma_start(out=damat[:], in_=_bc(da_dram[h:h + 1, :], 64))
                dab = hw.tile([64, N], F32, name="dab")
                nc.vector.tensor_scalar(out=dab[:], in0=damat[:], scalar1=gnb_sb[:, 0:1],
                                        op0=ALU.mult)
                nc.vector.tensor_tensor(out=t_t[:], in0=t_t[:], in1=dab[:], op=ALU.add)
            nc.vector.tensor_tensor(out=t_t[:], in0=t_t[:], in1=w_t[:], op=ALU.add)
            f2 = hw.tile([64, N], BF16, name="f2")
            nc.scalar.activation(out=f2[:], in_=t_t[:], func=AF.Gelu)

            # --- h2^T = W2^T f2^T (+b2); h2 token layout for pass B ---
            h2b = hw.tile([64, N], BF16, name="h2b")
            for ch in range(4):
                sl = slice(ch * 512, (ch + 1) * 512)
                ps = smps.tile([64, 512], F32, name="smp")
                nc.tensor.matmul(ps[:], gnw_sb[:, 1, :], f2[:, sl], start=True, stop=True)
                if has_gnn_b:
                    nc.vector.tensor_scalar(out=ps[:], in0=ps[:], scalar1=gnb_sb[:, 1:2],
                                            op0=ALU.add)
                nc.scalar.copy(out=h2b[:, sl], in_=ps[:])
            h2tok = hw.tile([128, NT, 64], BF16, name="h2tok")
            with tc.tile_pool(name="h2ps", bufs=2, space="PSUM") as h2ps:
                for quad in range(4):
                    ps = h2ps.tile([128, 256], BF16, name="h2p")
                    for q in range(4):
                        it = quad * 4 + q
                        nc.tensor.transpose(ps[:, q * 64:(q + 1) * 64],
                                            h2b[:, it * 128:(it + 1) * 128],
                                            ident_b[0:64, 0:64])
                    nc.vector.tensor_copy(
                        out=h2tok[:, quad * 4:(quad + 1) * 4, :],
                        in_=ps[:].rearrange("p (t d) -> p t d", t=4))

        # --- pass B: t4 = (E h2)^T * a + (h2^T * b + f2) ; f3 = gelu(t4) ---
        u_t = hw.tile([64, N], BF16, name="u_t")
        nc.vector.tensor_tensor(out=u_t[:], in0=h2b[:], in1=b_m, op=ALU.mult)
        nc.vector.tensor_tensor(out=u_t[:], in0=u_t[:], in1=f2[:], op=ALU.add)
        t4 = hw.tile([64, N], F32, name="t_t")   # reuse t_t slot
        with tc.tile_pool(name="psB", bufs=2, space="PSUM") as psBp:
            for ich in range(4):
                i0 = ich * 512
                ps = psBp.tile([64, 512], F32, name="psB")
                njt = min(NT, (ich + 1) * 4)
                for jt in range(njt):
                    rel = max(0, 128 * jt - i0)
                    ecol = ET_OFF[jt] + (i0 + rel - 128 * jt)
                    nc.tensor.matmul(ps[:, rel:512], h2tok[:, jt, :],
                                     et[:, ecol:ecol + (512 - rel)],
                                     start=(jt == 0), stop=(jt == njt - 1))
                nc.vector.tensor_tensor(out=t4[:, i0:i0 + 512], in0=ps[:],
                                        in1=a_m[:, i0:i0 + 512], op=ALU.mult)
        nc.vector.tensor_tensor(out=t4[:], in0=t4[:], in1=u_t[:], op=ALU.add)
        f3 = hw.tile([64, N], BF16, name="f3")
        nc.scalar.activation(out=f3[:], in_=t4[:], func=AF.Gelu)

        # --- gnn_out = lin_w^T f3 (+lin_b); out_h = EV*invD + gnn_out ---
        with tc.tile_pool(name="lnps", bufs=2, space="PSUM") as lnps:
            for ch in range(4):
                sl = slice(ch * 512, (ch + 1) * 512)
                ps = lnps.tile([64, 512], F32, name="lnp")
                nc.tensor.matmul(ps[:], linw_sb[:], f3[:, sl], start=True, stop=True)
                if has_lin_b:
                    nc.vector.tensor_scalar(out=ps[:], in0=ps[:], scalar1=linb_sb[:], op0=ALU.add)
                at = hw2.tile([64, 512], F32, name="at")
                nc.vector.tensor_tensor(out=at[:], in0=evt[:, sl], in1=invd_m[:, sl], op=ALU.mult)
                dst = out_pair[p][hsl, sl]
                if hh == 0:
                    nc.vector.tensor_tensor(out=dst, in0=ps[:], in1=at[:], op=ALU.add)
                else:
                    stg = hw2.tile([64, 512], BF16, name="stg")
                    nc.vector.tensor_tensor(out=stg[:], in0=ps[:], in1=at[:], op=ALU.add)
                    nc.sync.dma_start(out=dst, in_=stg[:])


# ---------------------------------------------------------------------------

_PROGRAM_CACHE = {}


def kernel(x, gamma_in, Wq, Wk, Wv, gnn_w, gnn_b, lin_w, lin_b, Wo, gamma_out):
    import ml_dtypes
    x = np.asarray(x, dtype=np.float32)
    gamma_in = np.asarray(gamma_in, dtype=np.float32)
    Wq = np.asarray(Wq, dtype=np.float32)
    Wk = np.asarray(Wk, dtype=np.float32)
    Wv = np.asarray(Wv, dtype=np.float32)
    gnn_w = np.asarray(gnn_w, dtype=np.float32)
    gnn_b = np.asarray(gnn_b, dtype=np.float32)
    lin_w = np.asarray(lin_w, dtype=np.float32)
    lin_b = np.asarray(lin_b, dtype=np.float32)
    Wo = np.asarray(Wo, dtype=np.float32)
    gamma_out = np.asarray(gamma_out, dtype=np.float32)

    key = (bool(np.any(gnn_b)), bool(np.any(lin_b)),
           bool(np.all(gamma_in == 1.0)), bool(np.all(gamma_out == 1.0)))
    if key not in _PROGRAM_CACHE:
        _PROGRAM_CACHE[key] = build_program(*key)
    nc = _PROGRAM_CACHE[key]

    in_maps = []
    for c in range(8):
        b, g = divmod(c, 4)
        cs = slice(g * 256, (g + 1) * 256)
        in_maps.append(dict(
            x_in=np.ascontiguousarray(x[b]),
            wq_in=np.ascontiguousarray(Wq[:, cs]),
            wk_in=np.ascontiguousarray(Wk[:, cs]),
            wv_in=np.ascontiguousarray(Wv[:, cs]),
            wo_in=np.ascontiguousarray(Wo[cs, :]).astype(ml_dtypes.bfloat16),
            gin_in=gamma_in[None, :],
            gout_in=gamma_out[None, :],
            gnw_in=gnn_w.astype(ml_dtypes.bfloat16),
            gnb_in=gnn_b,
            linw_in=lin_w.astype(ml_dtypes.bfloat16),
            linb_in=lin_b[None, :],
        ))
    trace = bool(int(os.environ.get("KERNEL_TRACE", "0")))
    res = run_bass_kernel_spmd(nc, in_maps, list(range(8)), trace=trace)
    if trace:
        kernel.last_exec_time_ns = res.exec_time_ns

    out = np.empty((B, N, DIM), dtype=np.float32)
    for c in range(8):
        b, g = divmod(c, 4)
        out[b, g * 512:(g + 1) * 512, :] = res.results[c]["y_out"]
    return out
